# revision 4
# baseline (speedup 1.0000x reference)
"""
Trainium2 Bass kernel v3 for Llama GQA decode attention (B=8, Q=4, H=4096,
32 Q-heads / 8 KV-heads, HD=128, S=4096 cached tokens, fp32 in/out).

Sharding: tensor-parallel over kv heads across 8 cores (core c owns kv head
c, its 4 q heads, Wo rows c*512..). Full output = host sum of per-core
partials (no collectives).

v3 (cost model: matmul cost = out_free_rows x pe_cycle; LDWEIGHTS free):
 - Out-proj flipped to Wo-STATIONARY: 32 j-tiles x 4 g matmuls of F=32
   (4096 PE rows) vs v2's 32 x F=512 (16384 rows). Output arrives as
   outT [j, tok]; the host transposes.
 - Per-batch KV DMA: kT_b (0.52MB) on the sync HWDGE queue, v8_b on the
   scalar queue, so batch b's attention starts the moment its K arrives.
   wo last in 8 chunks (4 per queue) chased by out-proj; out writes at the
   end of both queues.
 - Softmax denominator via 33 tiny fp8 ones-matmuls per batch on the PE
   (F=16 each) accumulating in PSUM, replacing the 1.07us/batch strided
   DVE tensor_reduce that sat on the exp->PV critical chain.
 - One all-batch V transpose matmul ([32,128]) instead of 8 per-batch.
Precision strategy (validated against the 2e-2 rel-err gate) as v2:
 fp8 (x32/x16 scales) Wq/Wk/Wv/hidden/K/V with V mean-extraction (PV
 computes sum_s (p-1)*V8; exact fp32 colsum added host-side), p via
 pd8=(p-1)*2048 fp8, Wo fp16.
"""

import os
import sys

sys.path.insert(0, "/opt/trn_rl_repo")

import math

import numpy as np

import concourse.bass as bass  # noqa: F401
import concourse.tile as tile
from concourse import bacc, bass_isa, bass_utils, mybir

# Problem constants (hardcoded per contract)
B, Q, H = 8, 4, 4096
NH, NKV, HD = 32, 8, 128
G = NH // NKV            # 4 query heads per kv head
S = 4096                 # cache token capacity actually used
TOK = B * Q              # 32 new tokens
GQ = G * Q               # 16 (head, query) pairs per batch
DC = G * HD              # 512 = per-core q/o head-dim slice
N_CORES = 8
SCALE = 1.0 / (HD ** 0.5)
KT = H // 128            # 32 contraction tiles over H
NJT = H // 128           # 32 output j-tiles
NWOC = 8                 # wo DMA chunks (4 j-tiles each)
JPC = NJT // NWOC        # 4 j-tiles per wo chunk

HS_S = 16.0              # hidden-state fp8 scale
W_S = 32.0               # Wq/Wk/Wv fp8 scale
KV_S = 16.0              # K/V cache fp8 scale
PQ = HS_S * W_S          # 512 = q/k projection psum scale
PD_S = 2048.0            # (p-1) fp8 scale
OSC = KV_S * PD_S        # 32768 = oT psum / colsum / den scale
C1 = SCALE / (KV_S * PQ)         # cached-score exp scale
C2 = SCALE / (PQ * PQ)           # new-token score exp scale
NEG = -1.0e30

FP32 = mybir.dt.float32
FP16 = mybir.dt.float16
FP8 = mybir.dt.float8e4
Exp = mybir.ActivationFunctionType.Exp
Copy = mybir.ActivationFunctionType.Copy
Mult = mybir.AluOpType.mult
Subtract = mybir.AluOpType.subtract
Add = mybir.AluOpType.add

HEAT_PRE = 12     # warm-up matmuls before the projection phase

# consts32 column map: [cos 0:32 | sin 32:64 | colsumB 64:192 | mnewT 192:208]
CS_COS, CS_SIN, CS_CSUM, CS_MNEW, CS_W = 0, 32, 64, 192, 208
# consts16 column map: [rt 0:128 | id128 128:256]
C16_RT, C16_ID, C16_W = 0, 128, 256


def _build_program(nts: tuple, rems: tuple):
    """Build + compile, specialized on per-batch cached-tile counts `nts`
    and boundary-tile valid-row counts `rems`."""
    nc = bacc.Bacc("TRN2", target_bir_lowering=False, debug=False,
                   num_devices=N_CORES)

    hs8_d = nc.dram_tensor("hs8", [128, KT, TOK], FP8, kind="ExternalInput").ap()
    wqkv_d = nc.dram_tensor("wqkv", [128, KT, DC + 2 * HD], FP8,
                            kind="ExternalInput").ap()
    # wo chunk-major: [128(d within g), chunk, g, 512 j] so every chunk is
    # a contiguous 4KB/partition DMA
    wo_d = nc.dram_tensor("wo", [128, NWOC, G, JPC * 128], FP16,
                          kind="ExternalInput").ap()
    kT_d = nc.dram_tensor("kT", [128, B, S], FP8, kind="ExternalInput").ap()
    v8_d = nc.dram_tensor("v8", [128, B, S // 128, HD], FP8,
                          kind="ExternalInput").ap()
    c32_d = nc.dram_tensor("c32", [128, CS_W], FP32, kind="ExternalInput").ap()
    c16_d = nc.dram_tensor("c16", [128, C16_W], FP16, kind="ExternalInput").ap()
    # transposed output: out[j-part, j-tile, tok]; host reassembles
    out_d = nc.dram_tensor("out", [128, NJT, TOK], FP32,
                           kind="ExternalOutput").ap()

    with tile.TileContext(nc) as tc:
        consts = tc.alloc_tile_pool(name="consts", bufs=1)
        wpool = tc.alloc_tile_pool(name="wtiles", bufs=1)
        kvpool = tc.alloc_tile_pool(name="kv", bufs=1)
        work = tc.alloc_tile_pool(name="work", bufs=1)
        ppool = tc.alloc_tile_pool(name="pbuf", bufs=2)
        pdpool = tc.alloc_tile_pool(name="pdbuf", bufs=B)
        pnpool = tc.alloc_tile_pool(name="pnbuf", bufs=B)
        ps_pj = tc.alloc_tile_pool(name="ps_pj", bufs=1, space="PSUM")
        ps_sc = tc.alloc_tile_pool(name="ps_sc", bufs=2, space="PSUM")
        ps_vs = tc.alloc_tile_pool(name="ps_vs", bufs=1, space="PSUM")
        ps_o = tc.alloc_tile_pool(name="ps_o", bufs=1, space="PSUM")
        ps_ot = tc.alloc_tile_pool(name="ps_ot", bufs=2, space="PSUM")

        # ---- SBUF tiles ----
        NWC = 4
        kk = KT // NWC
        hs8_sb = consts.tile([128, KT, TOK], FP8)
        c32_sb = consts.tile([128, CS_W], FP32)
        c16_sb = consts.tile([128, C16_W], FP16)
        wqkv_ts = [wpool.tile([128, kk, DC + 2 * HD], FP8, name=f"wqkv{i}")
                   for i in range(NWC)]
        wo_cs = [wpool.tile([128, G, JPC * 128], FP16, name=f"wo{i}")
                 for i in range(NWOC)]
        kT_bs = [kvpool.tile([128, S], FP8, name=f"kT_b{b}")
                 for b in range(B)]
        v8_bs = [kvpool.tile([128, S // 128, HD], FP8, name=f"v8_b{b}")
                 for b in range(B)]

        # ---- DMA starts. Queue plan (both HWDGE queues drain in parallel
        # at ~205GB/s each):
        #  sync:   hs8, wqkv0, wqkv2, kT b0..b7, wo c0,2,4,6, out x2
        #  scalar: c32, c16, wqkv1, wqkv3, v8 b0..b7, wo c1,3,5,7, out x2
        nc.sync.dma_start(out=hs8_sb, in_=hs8_d)
        nc.scalar.dma_start(out=c32_sb, in_=c32_d)
        nc.scalar.dma_start(out=c16_sb, in_=c16_d)
        eng = [nc.sync, nc.scalar]
        for i in range(NWC):
            eng[i % 2].dma_start(out=wqkv_ts[i],
                                 in_=wqkv_d[:, i * kk:(i + 1) * kk, :])
        for b in range(B):
            nc.sync.dma_start(out=kT_bs[b], in_=kT_d[:, b, :])
            nc.scalar.dma_start(out=v8_bs[b], in_=v8_d[:, b, :, :])
        for i in range(NWOC):
            eng[i % 2].dma_start(out=wo_cs[i], in_=wo_d[:, i, :, :])

        ones1r = consts.tile([1, 128], FP32)
        nc.vector.memset(ones1r, 1.0)
        lnb = consts.tile([Q, 1], FP32)
        nc.vector.memset(lnb, math.log(PD_S))
        heat16 = consts.tile([128, 128], FP16)
        nc.vector.memset(heat16, 0.001)
        ones8 = consts.tile([128, 1], FP8)
        nc.vector.memset(ones8, 1.0)
        ones16 = consts.tile([Q, 1], FP16)
        nc.vector.memset(ones16, 1.0)
        # dummy exp so the ACT table load happens while ACT is idle early
        dume = consts.tile([1, 1], FP32)
        nc.scalar.activation(dume, ones1r[0:1, 0:1], Exp)

        # PSUM: obank = [oT 0:128 | recbc 128:256 | den row 256:384]
        obank = ps_o.tile([128, 3 * B * GQ], FP32)
        den_ps = obank[0:1, 2 * B * GQ:3 * B * GQ]
        bankA = ps_pj.tile([128, 512], FP32, name="bankA")
        bankB = ps_pj.tile([128, 512], FP32, name="bankB")
        heat_ps = bankA[:, 0:128]

        def heat(n):
            for _ in range(n):
                nc.tensor.matmul(heat_ps, heat16, heat16, start=True,
                                 stop=True)

        heat(HEAT_PRE)

        rt = c16_sb[:, C16_RT:C16_RT + 128]
        id128 = c16_sb[:, C16_ID:C16_ID + 128]
        cosT = c32_sb[:, CS_COS:CS_COS + TOK]
        sinT = c32_sb[:, CS_SIN:CS_SIN + TOK]
        csumB = c32_sb[:, CS_CSUM:CS_CSUM + B * GQ]
        mnewT = c32_sb[0:Q, CS_MNEW:CS_MNEW + GQ]

        # ---- QKV projection, W-stationary: outputs arrive pre-transposed
        # [d, tok]. qT_ps[:, n, :]: n=0..3 q head-slices; 4 k; 5 v.
        qT_ps = bankB[:, 0:6 * TOK].rearrange("p (n t) -> p n t", n=6)
        for t in range(KT):
            wt = wqkv_ts[t // kk]
            tt = t % kk
            for n in range(6):
                nc.tensor.matmul(qT_ps[:, n, :],
                                 wt[:, tt, n * 128:(n + 1) * 128],
                                 hs8_sb[:, t, :],
                                 start=(t == 0 and n == 0),
                                 stop=(t == KT - 1 and n == 5))

        q0_sb = work.tile([128, G, TOK], FP16)
        nc.vector.tensor_copy(q0_sb, qT_ps[:, 0:G, :])
        k0_sb = work.tile([128, TOK], FP16)
        nc.vector.tensor_copy(k0_sb, qT_ps[:, G, :])
        vT_sb = work.tile([128, TOK], FP16)
        nc.vector.tensor_copy(vT_sb, qT_ps[:, G + 1, :])

        # ---- RoPE: rotate-half via rt matmul, combine on DVE ----
        q0_flat = q0_sb.rearrange("p g t -> p (g t)")
        qrot_ps = bankA[:, 0:G * TOK]
        nc.tensor.matmul(qrot_ps, rt, q0_flat, start=True, stop=True)
        krot_ps = bankA[:, G * TOK:5 * TOK]
        nc.tensor.matmul(krot_ps, rt, k0_sb, start=True, stop=True)

        def bcast_g(ap):  # [128, TOK] -> [128, (G, TOK)] with g-stride 0
            return bass.AP(tensor=ap.tensor, offset=ap.offset,
                           ap=[ap.ap[0], [0, G], ap.ap[-1]])

        qf8 = work.tile([128, G * TOK], FP8)   # 512*q_roped, [d, (g, b, qi)]
        tmpq = work.tile([128, G * TOK], FP32)
        nc.vector.tensor_mul(tmpq, q0_flat, bcast_g(cosT))
        nc.vector.tensor_mul(qf8, qrot_ps, bcast_g(sinT))
        nc.vector.tensor_add(qf8, qf8, tmpq)
        kf8 = work.tile([128, TOK], FP8)       # 512*k_roped, [d, (b, qi)]
        tmpk = work.tile([128, TOK], FP32)
        nc.vector.tensor_mul(tmpk, k0_sb, cosT)
        nc.vector.tensor_mul(kf8, krot_ps, sinT)
        nc.vector.tensor_add(kf8, kf8, tmpk)

        qf_v = qf8.rearrange("p (g b q) -> p g b q", g=G, b=B)

        # ---- new-token V: per-batch PE transpose -> [qi, d] fp16 x16 ----
        vnew_bs = []
        for b in range(B):
            vn_ps = ps_vs.tile([Q, 144], FP32, tag="vs",
                               name=f"vn{b}")[:, 0:128]
            nc.tensor.matmul(vn_ps, vT_sb[:, b * Q:(b + 1) * Q], id128,
                             start=True, stop=True)
            vnew = work.tile([Q, 128], FP16, name=f"vnew{b}")
            nc.scalar.activation(vnew, vn_ps, Copy, scale=KV_S / PQ)
            vnew_bs.append(vnew)

        # ---- attention: per batch emit scores_b (+ new-token scores);
        # den+PV of batch b-LAG interleaved so the scalar-exp -> DVE-pd8
        # round trip never stalls the in-order PE queue.
        den_f = work.tile([1, B * GQ], FP32)
        oT_ps = obank[:, 0:B * GQ]
        pd8_bs = []
        pnT_bs = []

        def emit_scores(b):
            nt = nts[b]
            pd8 = pdpool.tile([128, max(nt, 1) * GQ], FP8, tag="pd",
                              name=f"pd8_{b}")
            if nt > 0:
                kT_b = kT_bs[b]
                scT_ps = ps_sc.tile([128, max(nt, 1) * GQ], FP32, tag="sc")
                for t in range(nt):
                    nc.tensor.matmul(scT_ps[:, t * GQ:(t + 1) * GQ],
                                     kT_b[:, t * 128:(t + 1) * 128],
                                     qf_v[:, :, b, :],
                                     start=(t == 0), stop=(t == nt - 1))
                if rems[b] < 128:
                    # invalid tail rows -> score 0 -> p=1 -> pd=0 (exact:
                    # ln/colsum/den constants exclude them)
                    nc.vector.memset(
                        scT_ps[rems[b]:128, (nt - 1) * GQ:nt * GQ], 0.0)
                pT = ppool.tile([128, nt * GQ], FP32, tag="pT")
                nc.scalar.activation(pT, scT_ps[:, :nt * GQ], Exp, scale=C1)
                nc.vector.tensor_scalar(pd8, pT, PD_S, PD_S, Mult, Subtract)
            pd8_bs.append(pd8)

            snT_ps = ps_vs.tile([Q, 144], FP32, tag="vs",
                                name=f"sn{b}")[:, 128:144]
            nc.tensor.matmul(snT_ps, kf8[:, b * Q:(b + 1) * Q],
                             qf_v[:, :, b, :], start=True, stop=True)
            nc.vector.tensor_add(snT_ps, snT_ps, mnewT)
            pnT = pnpool.tile([Q, GQ], FP16, tag="pn", name=f"pnT{b}")
            nc.scalar.activation(pnT, snT_ps, Exp, bias=lnb, scale=C2)
            pnT_bs.append(pnT)

        def emit_dpv(b):
            nt = nts[b]
            # denominator: partition+tile sum of pd8 via tiny fp8 matmuls
            dreg = den_ps[0:1, b * GQ:(b + 1) * GQ]
            for t in range(nt):
                nc.tensor.matmul(dreg, ones8,
                                 pd8_bs[b][:, t * GQ:(t + 1) * GQ],
                                 start=(t == 0), stop=False)
            nc.tensor.matmul(dreg, ones16, pnT_bs[b],
                             start=(nt == 0), stop=True)
            ln = (nt - 1) * 128 + rems[b] if nt > 0 else 0
            nc.vector.tensor_scalar(den_f[0:1, b * GQ:(b + 1) * GQ], dreg,
                                    KV_S, OSC * ln, Mult, Add)
            # PV (V-stationary): oT dev-part accumulation
            oreg = oT_ps[:, b * GQ:(b + 1) * GQ]
            if nt > 0:
                v8_b = v8_bs[b]
                for t in range(nt):
                    nc.tensor.matmul(oreg, v8_b[:, t, :],
                                     pd8_bs[b][:, t * GQ:(t + 1) * GQ],
                                     start=(t == 0), stop=False)
            nc.tensor.matmul(oreg, vnew_bs[b], pnT_bs[b],
                             start=(nt == 0), stop=True)

        LAG = 2
        for b in range(B):
            emit_scores(b)
            if b >= LAG:
                emit_dpv(b - LAG)
        for b in range(B - LAG, B):
            emit_dpv(b)

        # ---- normalize: o = (oT + colsumB) / (OSC * den_true) ----
        rec_sb = work.tile([1, B * GQ], FP32)
        nc.vector.reciprocal(rec_sb, den_f)
        recbc_ps = obank[:, B * GQ:2 * B * GQ]
        nc.tensor.matmul(recbc_ps, ones1r, rec_sb, start=True, stop=True)
        recbc_sb = work.tile([128, B * GQ], FP32)
        nc.vector.tensor_copy(recbc_sb, recbc_ps)

        onum = work.tile([128, B * GQ], FP32)
        nc.vector.tensor_add(onum, oT_ps, csumB)
        onorm = work.tile([128, B * GQ], FP16)
        nc.vector.tensor_mul(onorm, onum, recbc_sb)
        oT_flat = work.tile([128, G * TOK], FP16)   # [d, (g, b, q)]
        nc.vector.tensor_copy(
            oT_flat.rearrange("p (g b q) -> p g b q", g=G, b=B),
            onorm.rearrange("p (b g q) -> p g b q", g=G, b=B))

        # ---- out-proj, Wo-stationary: per j-tile accumulate over g into
        # PSUM [j, tok]; chunks chase the wo DMA; host transposes ----
        outT_sb = work.tile([128, NJT, TOK], FP32)
        for n in range(NWOC):
            wt = wo_cs[n]
            ot_ps = ps_ot.tile([128, JPC, TOK], FP32, tag="ot")
            for jl in range(JPC):
                for g in range(G):
                    nc.tensor.matmul(ot_ps[:, jl, :],
                                     wt[:, g, jl * 128:(jl + 1) * 128],
                                     oT_flat[:, g * TOK:(g + 1) * TOK],
                                     start=(g == 0), stop=(g == G - 1))
            # alternate PSUM->SBUF copies across DVE and ACT
            dst = outT_sb[:, n * JPC:(n + 1) * JPC, :]
            if n % 2 == 0:
                nc.vector.tensor_copy(dst, ot_ps)
            else:
                nc.scalar.activation(dst, ot_ps, Copy)
            # 4 out chunks of 8 j-tiles, 2 per HWDGE queue, issued at the
            # queue tails (after all wo bytes) so they never block wo data
            if n % 2 == 1:
                half = (n - 1) * JPC
                eng[(n // 2) % 2].dma_start(
                    out=out_d[:, half:half + 2 * JPC, :],
                    in_=outT_sb[:, half:half + 2 * JPC, :])

        ps_ot.release()
        ps_o.release()
        ps_vs.release()
        ps_sc.release()
        ps_pj.release()
        pnpool.release()
        pdpool.release()
        ppool.release()
        work.release()
        kvpool.release()
        wpool.release()
        consts.release()

    nc.compile()
    return nc


_PROGRAM_CACHE: dict = {}


def _get_program(nts, rems):
    key = (tuple(nts), tuple(rems))
    if key not in _PROGRAM_CACHE:
        _PROGRAM_CACHE[key] = _build_program(tuple(nts), tuple(rems))
    return _PROGRAM_CACHE[key]


def _prep_inputs(hidden_states, cos, sin, Wq, Wk, Wv, Wo, K_cache, V_cache,
                 cache_lens):
    """Host-side shard prep. Returns (in_maps, nts, rems)."""
    f32 = np.float32
    f16 = np.float16
    f8 = mybir.dt.np(FP8)
    hs = np.asarray(hidden_states, dtype=f32).reshape(TOK, H)
    # hs8[p, t, n] = 16 * hs[n, t*128+p]
    hs8 = np.ascontiguousarray(
        (hs.T * HS_S).reshape(KT, 128, TOK).transpose(1, 0, 2)).astype(f8)
    cosT = np.asarray(cos, dtype=f32).reshape(TOK, HD).T    # [d, (b,qi)]
    sinT = np.asarray(sin, dtype=f32).reshape(TOK, HD).T

    lens = np.asarray(cache_lens, dtype=np.int64)
    nts, rems = [], []
    for b in range(B):
        ln = int(min(max(lens[b], 0), S))
        nt = (ln + 127) // 128
        rem = ln - (nt - 1) * 128 if nt > 0 else 128
        nts.append(nt)
        rems.append(rem)

    # rotate-half matrix, transposed for lhsT use (rot = rt.T @ x)
    R = np.zeros((HD, HD), dtype=f32)
    hh = HD // 2
    for dp in range(hh):
        R[dp, dp + hh] = -1.0
        R[dp + hh, dp] = 1.0
    c16 = np.zeros((128, C16_W), dtype=f16)
    c16[:, C16_RT:C16_RT + 128] = R.T
    c16[:, C16_ID:C16_ID + 128] = np.eye(128, dtype=f16)

    # new-token causal mask, [j, (g, qi)] layout: j visible iff j <= qi
    mnewT = np.zeros((Q, GQ), dtype=f32)
    for j in range(Q):
        for g in range(G):
            for qi in range(Q):
                if j > qi:
                    mnewT[j, g * Q + qi] = NEG

    wq = np.asarray(Wq, dtype=f32)
    wk = np.asarray(Wk, dtype=f32)
    wv = np.asarray(Wv, dtype=f32)
    wo = np.asarray(Wo, dtype=f32)
    Kc = np.asarray(K_cache, dtype=f32)
    Vc = np.asarray(V_cache, dtype=f32)

    in_maps = []
    for c in range(N_CORES):
        # wqkv8[p, t, :]: cols 0:512 Wq-slice, 512:640 Wk, 640:768 Wv (x32)
        wqkv = np.empty((128, KT, DC + 2 * HD), dtype=f32)
        wcat = np.concatenate(
            [wq[:, c * DC:(c + 1) * DC], wk[:, c * HD:(c + 1) * HD],
             wv[:, c * HD:(c + 1) * HD]], axis=1) * W_S     # [H, 768]
        wqkv[:] = wcat.reshape(KT, 128, DC + 2 * HD).transpose(1, 0, 2)
        # wo16[p, ch, g, jl] = Wo[c*512 + g*128 + p, ch*512 + jl]
        wo16 = np.ascontiguousarray(
            wo[c * DC:(c + 1) * DC, :].reshape(G, 128, NWOC, JPC * 128)
            .transpose(1, 2, 0, 3)).astype(f16)
        # kT8[p, b, s] = 16 * K_cache[b, s, c, p]
        kT8 = np.ascontiguousarray(
            (Kc[:, :S, c, :] * KV_S).transpose(2, 0, 1)).astype(f8)
        # v8[p, b, t, d] = 16 * V_cache[b, t*128+p, c, d]
        v8 = np.ascontiguousarray(
            (Vc[:, :S, c, :] * KV_S).reshape(B, S // 128, 128, HD)
            .transpose(2, 0, 1, 3)).astype(f8)

        c32 = np.zeros((128, CS_W), dtype=f32)
        c32[:, CS_COS:CS_COS + TOK] = cosT
        c32[:, CS_SIN:CS_SIN + TOK] = sinT
        # colsumB[d, b*16+i] = OSC * sum_{s<ln_b} V_cache[b, s, c, d]
        for b in range(B):
            ln = (nts[b] - 1) * 128 + rems[b] if nts[b] > 0 else 0
            csum = Vc[b, :ln, c, :].astype(np.float64).sum(axis=0)
            c32[:, CS_CSUM + b * GQ:CS_CSUM + (b + 1) * GQ] = (
                OSC * csum.astype(f32))[:, None]
        c32[0:Q, CS_MNEW:CS_MNEW + GQ] = mnewT

        in_maps.append(dict(hs8=hs8, wqkv=wqkv.astype(f8), wo=wo16, kT=kT8,
                            v8=v8, c32=c32, c16=c16))
    return in_maps, nts, rems


def _install_axon_ntff_hook():
    """The agent image's antenv lacks axon_hooks; recreate the NTFF profile
    hook via ctypes against libaxon_pjrt.so so trace=True yields exec times."""
    try:
        from antenv.axon_hooks import get_axon_ntff_profile_hook  # noqa: F401
        return
    except ImportError:
        pass
    import contextlib
    import ctypes
    import types

    so_path = "/opt/axon/libaxon_pjrt.so"
    try:
        lib = ctypes.CDLL(so_path)
    except OSError:
        return
    if not hasattr(lib, "axon_start_nrt_profile"):
        return
    lib.axon_start_nrt_profile.argtypes = [ctypes.POINTER(ctypes.c_int64),
                                           ctypes.c_size_t]
    lib.axon_start_nrt_profile.restype = ctypes.c_int64
    lib.axon_stop_nrt_profile.argtypes = [ctypes.c_char_p]
    lib.axon_stop_nrt_profile.restype = ctypes.c_int64

    @contextlib.contextmanager
    def _hook(output_dir, device_ids):
        import jax
        jax.devices()
        if device_ids:
            ids = (ctypes.c_int64 * len(device_ids))(*device_ids)
            rc = lib.axon_start_nrt_profile(ids, len(device_ids))
        else:
            rc = lib.axon_start_nrt_profile(None, 0)
        if rc != 0:
            raise RuntimeError(f"axon_start_nrt_profile rc={rc}")
        try:
            yield
        finally:
            n = lib.axon_stop_nrt_profile(str(output_dir).encode())
            if n <= 0:
                print(f"profile: rc={n} writing to {output_dir}",
                      file=sys.stderr)

    import antenv
    mod = types.ModuleType("antenv.axon_hooks")
    mod.get_axon_ntff_profile_hook = lambda: _hook
    mod.set_axon_ntff_profile_hook = lambda h: None
    sys.modules["antenv.axon_hooks"] = mod
    antenv.axon_hooks = mod


_LAST_RESULTS = {}


def kernel(hidden_states, cos, sin, Wq, Wk, Wv, Wo, K_cache, V_cache,
           cache_lens):
    in_maps, nts, rems = _prep_inputs(hidden_states, cos, sin, Wq, Wk, Wv,
                                      Wo, K_cache, V_cache, cache_lens)
    nc = _get_program(nts, rems)

    trace = bool(int(os.environ.get("BASS_KERNEL_TRACE", "0")))
    if trace:
        _install_axon_ntff_hook()
    res = bass_utils.run_bass_kernel_spmd(
        nc, in_maps, core_ids=list(range(N_CORES)), trace=trace)
    _LAST_RESULTS["res"] = res

    total = np.zeros((H, TOK), dtype=np.float64)
    for c in range(N_CORES):
        o = res.results[c]["out"]            # [128, NJT, TOK]
        total += o.transpose(1, 0, 2).reshape(H, TOK).astype(np.float64)
    return total.T.astype(np.float32).reshape(B, Q, H)


# revision 6
# speedup vs baseline: 1.0438x; 1.0438x over previous
"""
Trainium2 Bass kernel v3 for Llama GQA decode attention (B=8, Q=4, H=4096,
32 Q-heads / 8 KV-heads, HD=128, S=4096 cached tokens, fp32 in/out).

Sharding: tensor-parallel over kv heads across 8 cores (core c owns kv head
c, its 4 q heads, Wo rows c*512..). Full output = host sum of per-core
partials (no collectives).

v3 (cost model: matmul cost = out_free_rows x pe_cycle; LDWEIGHTS free):
 - Out-proj flipped to Wo-STATIONARY: 32 j-tiles x 4 g matmuls of F=32
   (4096 PE rows) vs v2's 32 x F=512 (16384 rows). Output arrives as
   outT [j, tok]; the host transposes.
 - Per-batch KV DMA: kT_b (0.52MB) on the sync HWDGE queue, v8_b on the
   scalar queue, so batch b's attention starts the moment its K arrives.
   wo last in 8 chunks (4 per queue) chased by out-proj; out writes at the
   end of both queues.
 - Softmax denominator via 33 tiny fp8 ones-matmuls per batch on the PE
   (F=16 each) accumulating in PSUM, replacing the 1.07us/batch strided
   DVE tensor_reduce that sat on the exp->PV critical chain.
 - One all-batch V transpose matmul ([32,128]) instead of 8 per-batch.
Precision strategy (validated against the 2e-2 rel-err gate) as v2:
 fp8 (x32/x16 scales) Wq/Wk/Wv/hidden/K/V with V mean-extraction (PV
 computes sum_s (p-1)*V8; exact fp32 colsum added host-side), p via
 pd8=(p-1)*2048 fp8, Wo fp16.
"""

import os
import sys

sys.path.insert(0, "/opt/trn_rl_repo")

import math

import numpy as np

import concourse.bass as bass  # noqa: F401
import concourse.tile as tile
from concourse import bacc, bass_isa, bass_utils, mybir

# Problem constants (hardcoded per contract)
B, Q, H = 8, 4, 4096
NH, NKV, HD = 32, 8, 128
G = NH // NKV            # 4 query heads per kv head
S = 4096                 # cache token capacity actually used
TOK = B * Q              # 32 new tokens
GQ = G * Q               # 16 (head, query) pairs per batch
DC = G * HD              # 512 = per-core q/o head-dim slice
N_CORES = 8
SCALE = 1.0 / (HD ** 0.5)
KT = H // 128            # 32 contraction tiles over H
NJT = H // 128           # 32 output j-tiles
NWOC = 8                 # wo DMA chunks (4 j-tiles each)
JPC = NJT // NWOC        # 4 j-tiles per wo chunk

HS_S = 16.0              # hidden-state fp8 scale
W_S = 32.0               # Wq/Wk/Wv fp8 scale
KV_S = 16.0              # K/V cache fp8 scale
PQ = HS_S * W_S          # 512 = q/k projection psum scale
PD_S = 2048.0            # (p-1) fp8 scale
OSC = KV_S * PD_S        # 32768 = oT psum / colsum / den scale
C1 = SCALE / (KV_S * PQ)         # cached-score exp scale
C2 = SCALE / (PQ * PQ)           # new-token score exp scale
NEG = -1.0e30

FP32 = mybir.dt.float32
FP16 = mybir.dt.float16
FP8 = mybir.dt.float8e4
Exp = mybir.ActivationFunctionType.Exp
Copy = mybir.ActivationFunctionType.Copy
Mult = mybir.AluOpType.mult
Subtract = mybir.AluOpType.subtract
Add = mybir.AluOpType.add

HEAT_PRE = 12     # warm-up matmuls before the projection phase

# consts32 column map: [cos 0:32 | sin 32:64 | colsumB 64:192 | mnewT 192:208]
CS_COS, CS_SIN, CS_CSUM, CS_MNEW, CS_W = 0, 32, 64, 192, 208
# consts16 column map: [rt 0:128 | id128 128:256]
C16_RT, C16_ID, C16_W = 0, 128, 256


def _build_program(nts: tuple, rems: tuple):
    """Build + compile, specialized on per-batch cached-tile counts `nts`
    and boundary-tile valid-row counts `rems`."""
    nc = bacc.Bacc("TRN2", target_bir_lowering=False, debug=False,
                   num_devices=N_CORES)

    hs8_d = nc.dram_tensor("hs8", [128, KT, TOK], FP8, kind="ExternalInput").ap()
    wqkv_d = nc.dram_tensor("wqkv", [128, KT, DC + 2 * HD], FP8,
                            kind="ExternalInput").ap()
    # wo chunk-major: [128(d within g), chunk, g, 512 j] so every chunk is
    # a contiguous 4KB/partition DMA
    wo_d = nc.dram_tensor("wo", [128, NWOC, G, JPC * 128], FP16,
                          kind="ExternalInput").ap()
    kT_d = nc.dram_tensor("kT", [128, B, S], FP8, kind="ExternalInput").ap()
    v8_d = nc.dram_tensor("v8", [128, B, S // 128, HD], FP8,
                          kind="ExternalInput").ap()
    c32_d = nc.dram_tensor("c32", [128, CS_W], FP32, kind="ExternalInput").ap()
    c16_d = nc.dram_tensor("c16", [128, C16_W], FP16, kind="ExternalInput").ap()
    # transposed output: out[j-part, j-tile, tok]; host reassembles
    out_d = nc.dram_tensor("out", [128, NJT, TOK], FP32,
                           kind="ExternalOutput").ap()

    with tile.TileContext(nc) as tc:
        consts = tc.alloc_tile_pool(name="consts", bufs=1)
        wpool = tc.alloc_tile_pool(name="wtiles", bufs=1)
        kvpool = tc.alloc_tile_pool(name="kv", bufs=1)
        work = tc.alloc_tile_pool(name="work", bufs=1)
        ppool = tc.alloc_tile_pool(name="pbuf", bufs=2)
        pdpool = tc.alloc_tile_pool(name="pdbuf", bufs=B)
        pnpool = tc.alloc_tile_pool(name="pnbuf", bufs=B)
        ps_pj = tc.alloc_tile_pool(name="ps_pj", bufs=1, space="PSUM")
        ps_sc = tc.alloc_tile_pool(name="ps_sc", bufs=2, space="PSUM")
        ps_vs = tc.alloc_tile_pool(name="ps_vs", bufs=1, space="PSUM")
        ps_o = tc.alloc_tile_pool(name="ps_o", bufs=1, space="PSUM")
        ps_ot = tc.alloc_tile_pool(name="ps_ot", bufs=2, space="PSUM")

        # ---- SBUF tiles ----
        NWC = 4
        kk = KT // NWC
        hs8_sb = consts.tile([128, KT, TOK], FP8)
        c32_sb = consts.tile([128, CS_W], FP32)
        c16_sb = consts.tile([128, C16_W], FP16)
        wqkv_ts = [wpool.tile([128, kk, DC + 2 * HD], FP8, name=f"wqkv{i}")
                   for i in range(NWC)]
        wo_cs = [wpool.tile([128, G, JPC * 128], FP16, name=f"wo{i}")
                 for i in range(NWOC)]
        kT_bs = [kvpool.tile([128, S], FP8, name=f"kT_b{b}")
                 for b in range(B)]
        v8_g2 = [kvpool.tile([128, B // 2, S // 128, HD], FP8,
                             name=f"v8_g{i}") for i in range(2)]
        v8_bs = [v8_g2[b // 4][:, b % 4] for b in range(B)]

        # ---- DMA starts. Each HWDGE queue has only 4 rotating completion
        # semaphores; a trigger instruction BLOCKS its engine until the
        # transfer two generations back completes. The ACT engine must stay
        # responsive for the per-batch exp chain, so the scalar queue gets
        # exactly 8 upfront triggers (all gen-2 waits land on small early
        # transfers); wo c5/c7 triggers are interleaved after exp batches.
        # Sync (otherwise idle) takes the fine-grained kT chase.
        #  sync:   hs8, wqkv0, wqkv2, kT b0..b7, wo c0,2,4,6 [+ out x2]
        #  scalar: c32, c16, wqkv1, wqkv3, v8 b0-3, v8 b4-7, wo c1, c3
        #          ... exp b0..b3 ... wo c5 ... exp b4..b7 ... wo c7 [+ out x2]
        nc.sync.dma_start(out=hs8_sb, in_=hs8_d)
        nc.scalar.dma_start(out=c32_sb, in_=c32_d)
        nc.scalar.dma_start(out=c16_sb, in_=c16_d)
        eng = [nc.sync, nc.scalar]
        for i in range(NWC):
            eng[i % 2].dma_start(out=wqkv_ts[i],
                                 in_=wqkv_d[:, i * kk:(i + 1) * kk, :])
        for b in range(B):
            nc.sync.dma_start(out=kT_bs[b], in_=kT_d[:, b, :])
        for i in range(2):
            nc.scalar.dma_start(out=v8_g2[i],
                                in_=v8_d[:, 4 * i:4 * i + 4, :, :])
        for i in (0, 2, 4, 6):
            nc.sync.dma_start(out=wo_cs[i], in_=wo_d[:, i, :, :])
        for i in (1, 3):
            nc.scalar.dma_start(out=wo_cs[i], in_=wo_d[:, i, :, :])

        ones1r = consts.tile([1, 128], FP32)
        nc.vector.memset(ones1r, 1.0)
        lnb = consts.tile([Q, 1], FP32)
        nc.vector.memset(lnb, math.log(PD_S))
        heat16 = consts.tile([128, 128], FP16)
        nc.vector.memset(heat16, 0.001)
        ones8 = consts.tile([128, 1], FP8)
        nc.vector.memset(ones8, 1.0)
        ones16 = consts.tile([Q, 1], FP16)
        nc.vector.memset(ones16, 1.0)
        # dummy exp so the ACT table load happens while ACT is idle early
        dume = consts.tile([1, 1], FP32)
        nc.scalar.activation(dume, ones1r[0:1, 0:1], Exp)

        # PSUM: obank = [oT 0:128 | recbc 128:256 | den row 256:384]
        obank = ps_o.tile([128, 3 * B * GQ], FP32)
        den_ps = obank[0:1, 2 * B * GQ:3 * B * GQ]
        bankA = ps_pj.tile([128, 512], FP32, name="bankA")
        bankB = ps_pj.tile([128, 512], FP32, name="bankB")
        heat_ps = bankA[:, 0:128]

        def heat(n):
            for _ in range(n):
                nc.tensor.matmul(heat_ps, heat16, heat16, start=True,
                                 stop=True)

        heat(HEAT_PRE)

        rt = c16_sb[:, C16_RT:C16_RT + 128]
        id128 = c16_sb[:, C16_ID:C16_ID + 128]
        cosT = c32_sb[:, CS_COS:CS_COS + TOK]
        sinT = c32_sb[:, CS_SIN:CS_SIN + TOK]
        csumB = c32_sb[:, CS_CSUM:CS_CSUM + B * GQ]
        mnewT = c32_sb[0:Q, CS_MNEW:CS_MNEW + GQ]

        # ---- QKV projection, W-stationary: outputs arrive pre-transposed
        # [d, tok]. qT_ps[:, n, :]: n=0..3 q head-slices; 4 k; 5 v.
        qT_ps = bankB[:, 0:6 * TOK].rearrange("p (n t) -> p n t", n=6)
        for t in range(KT):
            wt = wqkv_ts[t // kk]
            tt = t % kk
            for n in range(6):
                nc.tensor.matmul(qT_ps[:, n, :],
                                 wt[:, tt, n * 128:(n + 1) * 128],
                                 hs8_sb[:, t, :],
                                 start=(t == 0 and n == 0),
                                 stop=(t == KT - 1 and n == 5))

        q0_sb = work.tile([128, G, TOK], FP16)
        nc.vector.tensor_copy(q0_sb, qT_ps[:, 0:G, :])
        k0_sb = work.tile([128, TOK], FP16)
        nc.vector.tensor_copy(k0_sb, qT_ps[:, G, :])
        vT_sb = work.tile([128, TOK], FP16)
        nc.vector.tensor_copy(vT_sb, qT_ps[:, G + 1, :])

        # ---- RoPE: rotate-half via rt matmul, combine on DVE ----
        q0_flat = q0_sb.rearrange("p g t -> p (g t)")
        qrot_ps = bankA[:, 0:G * TOK]
        nc.tensor.matmul(qrot_ps, rt, q0_flat, start=True, stop=True)
        krot_ps = bankA[:, G * TOK:5 * TOK]
        nc.tensor.matmul(krot_ps, rt, k0_sb, start=True, stop=True)

        def bcast_g(ap):  # [128, TOK] -> [128, (G, TOK)] with g-stride 0
            return bass.AP(tensor=ap.tensor, offset=ap.offset,
                           ap=[ap.ap[0], [0, G], ap.ap[-1]])

        qf8 = work.tile([128, G * TOK], FP8)   # 512*q_roped, [d, (g, b, qi)]
        tmpq = work.tile([128, G * TOK], FP32)
        nc.vector.tensor_mul(tmpq, q0_flat, bcast_g(cosT))
        nc.vector.tensor_mul(qf8, qrot_ps, bcast_g(sinT))
        nc.vector.tensor_add(qf8, qf8, tmpq)
        kf8 = work.tile([128, TOK], FP8)       # 512*k_roped, [d, (b, qi)]
        tmpk = work.tile([128, TOK], FP32)
        nc.vector.tensor_mul(tmpk, k0_sb, cosT)
        nc.vector.tensor_mul(kf8, krot_ps, sinT)
        nc.vector.tensor_add(kf8, kf8, tmpk)

        qf_v = qf8.rearrange("p (g b q) -> p g b q", g=G, b=B)

        # ---- new-token V: per-batch PE transpose -> [qi, d] fp16 x16 ----
        vnew_bs = []
        for b in range(B):
            vn_ps = ps_vs.tile([Q, 144], FP32, tag="vs",
                               name=f"vn{b}")[:, 0:128]
            nc.tensor.matmul(vn_ps, vT_sb[:, b * Q:(b + 1) * Q], id128,
                             start=True, stop=True)
            vnew = work.tile([Q, 128], FP16, name=f"vnew{b}")
            nc.scalar.activation(vnew, vn_ps, Copy, scale=KV_S / PQ)
            vnew_bs.append(vnew)

        # ---- attention: per batch emit scores_b (+ new-token scores);
        # den+PV of batch b-LAG interleaved so the scalar-exp -> DVE-pd8
        # round trip never stalls the in-order PE queue.
        den_f = work.tile([1, B * GQ], FP32)
        oT_ps = obank[:, 0:B * GQ]
        pd8_bs = []
        pnT_bs = []

        def emit_scores(b):
            nt = nts[b]
            pd8 = pdpool.tile([128, max(nt, 1) * GQ], FP8, tag="pd",
                              name=f"pd8_{b}")
            if nt > 0:
                kT_b = kT_bs[b]
                scT_ps = ps_sc.tile([128, max(nt, 1) * GQ], FP32, tag="sc")
                for t in range(nt):
                    nc.tensor.matmul(scT_ps[:, t * GQ:(t + 1) * GQ],
                                     kT_b[:, t * 128:(t + 1) * 128],
                                     qf_v[:, :, b, :],
                                     start=(t == 0), stop=(t == nt - 1))
                if rems[b] < 128:
                    # invalid tail rows -> score 0 -> p=1 -> pd=0 (exact:
                    # ln/colsum/den constants exclude them)
                    nc.vector.memset(
                        scT_ps[rems[b]:128, (nt - 1) * GQ:nt * GQ], 0.0)
                pT = ppool.tile([128, nt * GQ], FP32, tag="pT")
                nc.scalar.activation(pT, scT_ps[:, :nt * GQ], Exp, scale=C1)
                nc.vector.tensor_scalar(pd8, pT, PD_S, PD_S, Mult, Subtract)
            pd8_bs.append(pd8)

            snT_ps = ps_vs.tile([Q, 144], FP32, tag="vs",
                                name=f"sn{b}")[:, 128:144]
            nc.tensor.matmul(snT_ps, kf8[:, b * Q:(b + 1) * Q],
                             qf_v[:, :, b, :], start=True, stop=True)
            nc.vector.tensor_add(snT_ps, snT_ps, mnewT)
            pnT = pnpool.tile([Q, GQ], FP16, tag="pn", name=f"pnT{b}")
            nc.scalar.activation(pnT, snT_ps, Exp, bias=lnb, scale=C2)
            pnT_bs.append(pnT)

        def emit_dpv(b):
            nt = nts[b]
            # denominator: partition+tile sum of pd8 via tiny fp8 matmuls
            dreg = den_ps[0:1, b * GQ:(b + 1) * GQ]
            for t in range(nt):
                nc.tensor.matmul(dreg, ones8,
                                 pd8_bs[b][:, t * GQ:(t + 1) * GQ],
                                 start=(t == 0), stop=False)
            nc.tensor.matmul(dreg, ones16, pnT_bs[b],
                             start=(nt == 0), stop=True)
            ln = (nt - 1) * 128 + rems[b] if nt > 0 else 0
            nc.vector.tensor_scalar(den_f[0:1, b * GQ:(b + 1) * GQ], dreg,
                                    KV_S, OSC * ln, Mult, Add)
            # PV (V-stationary): oT dev-part accumulation
            oreg = oT_ps[:, b * GQ:(b + 1) * GQ]
            if nt > 0:
                v8_b = v8_bs[b]
                for t in range(nt):
                    nc.tensor.matmul(oreg, v8_b[:, t, :],
                                     pd8_bs[b][:, t * GQ:(t + 1) * GQ],
                                     start=(t == 0), stop=False)
            nc.tensor.matmul(oreg, vnew_bs[b], pnT_bs[b],
                             start=(nt == 0), stop=True)

        LAG = 4
        for b in range(B):
            emit_scores(b)
            # late wo triggers: placed here so their backpressure waits
            # never block exps that are ready to run
            if b == 3:
                nc.scalar.dma_start(out=wo_cs[5], in_=wo_d[:, 5, :, :])
            if b == 7:
                nc.scalar.dma_start(out=wo_cs[7], in_=wo_d[:, 7, :, :])
            if b >= LAG:
                emit_dpv(b - LAG)
        for b in range(B - LAG, B):
            emit_dpv(b)

        # ---- normalize: o = (oT + colsumB) / (OSC * den_true) ----
        rec_sb = work.tile([1, B * GQ], FP32)
        nc.vector.reciprocal(rec_sb, den_f)
        recbc_ps = obank[:, B * GQ:2 * B * GQ]
        nc.tensor.matmul(recbc_ps, ones1r, rec_sb, start=True, stop=True)
        recbc_sb = work.tile([128, B * GQ], FP32)
        nc.vector.tensor_copy(recbc_sb, recbc_ps)

        onum = work.tile([128, B * GQ], FP32)
        nc.vector.tensor_add(onum, oT_ps, csumB)
        onorm = work.tile([128, B * GQ], FP16)
        nc.vector.tensor_mul(onorm, onum, recbc_sb)
        oT_flat = work.tile([128, G * TOK], FP16)   # [d, (g, b, q)]
        nc.vector.tensor_copy(
            oT_flat.rearrange("p (g b q) -> p g b q", g=G, b=B),
            onorm.rearrange("p (b g q) -> p g b q", g=G, b=B))

        # ---- out-proj, Wo-stationary: per j-tile accumulate over g into
        # PSUM [j, tok]; chunks chase the wo DMA; host transposes ----
        outT_sb = work.tile([128, NJT, TOK], FP32)
        for n in range(NWOC):
            wt = wo_cs[n]
            ot_ps = ps_ot.tile([128, JPC, TOK], FP32, tag="ot")
            for jl in range(JPC):
                for g in range(G):
                    nc.tensor.matmul(ot_ps[:, jl, :],
                                     wt[:, g, jl * 128:(jl + 1) * 128],
                                     oT_flat[:, g * TOK:(g + 1) * TOK],
                                     start=(g == 0), stop=(g == G - 1))
            # alternate PSUM->SBUF copies across DVE and ACT
            dst = outT_sb[:, n * JPC:(n + 1) * JPC, :]
            if n % 2 == 0:
                nc.vector.tensor_copy(dst, ot_ps)
            else:
                nc.scalar.activation(dst, ot_ps, Copy)
            # 4 out chunks of 8 j-tiles, 2 per HWDGE queue, issued at the
            # queue tails (after all wo bytes) so they never block wo data
            if n % 2 == 1:
                half = (n - 1) * JPC
                eng[(n // 2) % 2].dma_start(
                    out=out_d[:, half:half + 2 * JPC, :],
                    in_=outT_sb[:, half:half + 2 * JPC, :])

        ps_ot.release()
        ps_o.release()
        ps_vs.release()
        ps_sc.release()
        ps_pj.release()
        pnpool.release()
        pdpool.release()
        ppool.release()
        work.release()
        kvpool.release()
        wpool.release()
        consts.release()

    nc.compile()
    return nc


_PROGRAM_CACHE: dict = {}


def _get_program(nts, rems):
    key = (tuple(nts), tuple(rems))
    if key not in _PROGRAM_CACHE:
        _PROGRAM_CACHE[key] = _build_program(tuple(nts), tuple(rems))
    return _PROGRAM_CACHE[key]


def _prep_inputs(hidden_states, cos, sin, Wq, Wk, Wv, Wo, K_cache, V_cache,
                 cache_lens):
    """Host-side shard prep. Returns (in_maps, nts, rems)."""
    f32 = np.float32
    f16 = np.float16
    f8 = mybir.dt.np(FP8)
    hs = np.asarray(hidden_states, dtype=f32).reshape(TOK, H)
    # hs8[p, t, n] = 16 * hs[n, t*128+p]
    hs8 = np.ascontiguousarray(
        (hs.T * HS_S).reshape(KT, 128, TOK).transpose(1, 0, 2)).astype(f8)
    cosT = np.asarray(cos, dtype=f32).reshape(TOK, HD).T    # [d, (b,qi)]
    sinT = np.asarray(sin, dtype=f32).reshape(TOK, HD).T

    lens = np.asarray(cache_lens, dtype=np.int64)
    nts, rems = [], []
    for b in range(B):
        ln = int(min(max(lens[b], 0), S))
        nt = (ln + 127) // 128
        rem = ln - (nt - 1) * 128 if nt > 0 else 128
        nts.append(nt)
        rems.append(rem)

    # rotate-half matrix, transposed for lhsT use (rot = rt.T @ x)
    R = np.zeros((HD, HD), dtype=f32)
    hh = HD // 2
    for dp in range(hh):
        R[dp, dp + hh] = -1.0
        R[dp + hh, dp] = 1.0
    c16 = np.zeros((128, C16_W), dtype=f16)
    c16[:, C16_RT:C16_RT + 128] = R.T
    c16[:, C16_ID:C16_ID + 128] = np.eye(128, dtype=f16)

    # new-token causal mask, [j, (g, qi)] layout: j visible iff j <= qi
    mnewT = np.zeros((Q, GQ), dtype=f32)
    for j in range(Q):
        for g in range(G):
            for qi in range(Q):
                if j > qi:
                    mnewT[j, g * Q + qi] = NEG

    wq = np.asarray(Wq, dtype=f32)
    wk = np.asarray(Wk, dtype=f32)
    wv = np.asarray(Wv, dtype=f32)
    wo = np.asarray(Wo, dtype=f32)
    Kc = np.asarray(K_cache, dtype=f32)
    Vc = np.asarray(V_cache, dtype=f32)

    in_maps = []
    for c in range(N_CORES):
        # wqkv8[p, t, :]: cols 0:512 Wq-slice, 512:640 Wk, 640:768 Wv (x32)
        wqkv = np.empty((128, KT, DC + 2 * HD), dtype=f32)
        wcat = np.concatenate(
            [wq[:, c * DC:(c + 1) * DC], wk[:, c * HD:(c + 1) * HD],
             wv[:, c * HD:(c + 1) * HD]], axis=1) * W_S     # [H, 768]
        wqkv[:] = wcat.reshape(KT, 128, DC + 2 * HD).transpose(1, 0, 2)
        # wo16[p, ch, g, jl] = Wo[c*512 + g*128 + p, ch*512 + jl]
        wo16 = np.ascontiguousarray(
            wo[c * DC:(c + 1) * DC, :].reshape(G, 128, NWOC, JPC * 128)
            .transpose(1, 2, 0, 3)).astype(f16)
        # kT8[p, b, s] = 16 * K_cache[b, s, c, p]
        kT8 = np.ascontiguousarray(
            (Kc[:, :S, c, :] * KV_S).transpose(2, 0, 1)).astype(f8)
        # v8[p, b, t, d] = 16 * V_cache[b, t*128+p, c, d]
        v8 = np.ascontiguousarray(
            (Vc[:, :S, c, :] * KV_S).reshape(B, S // 128, 128, HD)
            .transpose(2, 0, 1, 3)).astype(f8)

        c32 = np.zeros((128, CS_W), dtype=f32)
        c32[:, CS_COS:CS_COS + TOK] = cosT
        c32[:, CS_SIN:CS_SIN + TOK] = sinT
        # colsumB[d, b*16+i] = OSC * sum_{s<ln_b} V_cache[b, s, c, d]
        for b in range(B):
            ln = (nts[b] - 1) * 128 + rems[b] if nts[b] > 0 else 0
            csum = Vc[b, :ln, c, :].astype(np.float64).sum(axis=0)
            c32[:, CS_CSUM + b * GQ:CS_CSUM + (b + 1) * GQ] = (
                OSC * csum.astype(f32))[:, None]
        c32[0:Q, CS_MNEW:CS_MNEW + GQ] = mnewT

        in_maps.append(dict(hs8=hs8, wqkv=wqkv.astype(f8), wo=wo16, kT=kT8,
                            v8=v8, c32=c32, c16=c16))
    return in_maps, nts, rems


def _install_axon_ntff_hook():
    """The agent image's antenv lacks axon_hooks; recreate the NTFF profile
    hook via ctypes against libaxon_pjrt.so so trace=True yields exec times."""
    try:
        from antenv.axon_hooks import get_axon_ntff_profile_hook  # noqa: F401
        return
    except ImportError:
        pass
    import contextlib
    import ctypes
    import types

    so_path = "/opt/axon/libaxon_pjrt.so"
    try:
        lib = ctypes.CDLL(so_path)
    except OSError:
        return
    if not hasattr(lib, "axon_start_nrt_profile"):
        return
    lib.axon_start_nrt_profile.argtypes = [ctypes.POINTER(ctypes.c_int64),
                                           ctypes.c_size_t]
    lib.axon_start_nrt_profile.restype = ctypes.c_int64
    lib.axon_stop_nrt_profile.argtypes = [ctypes.c_char_p]
    lib.axon_stop_nrt_profile.restype = ctypes.c_int64

    @contextlib.contextmanager
    def _hook(output_dir, device_ids):
        import jax
        jax.devices()
        if device_ids:
            ids = (ctypes.c_int64 * len(device_ids))(*device_ids)
            rc = lib.axon_start_nrt_profile(ids, len(device_ids))
        else:
            rc = lib.axon_start_nrt_profile(None, 0)
        if rc != 0:
            raise RuntimeError(f"axon_start_nrt_profile rc={rc}")
        try:
            yield
        finally:
            n = lib.axon_stop_nrt_profile(str(output_dir).encode())
            if n <= 0:
                print(f"profile: rc={n} writing to {output_dir}",
                      file=sys.stderr)

    import antenv
    mod = types.ModuleType("antenv.axon_hooks")
    mod.get_axon_ntff_profile_hook = lambda: _hook
    mod.set_axon_ntff_profile_hook = lambda h: None
    sys.modules["antenv.axon_hooks"] = mod
    antenv.axon_hooks = mod


_LAST_RESULTS = {}


def kernel(hidden_states, cos, sin, Wq, Wk, Wv, Wo, K_cache, V_cache,
           cache_lens):
    in_maps, nts, rems = _prep_inputs(hidden_states, cos, sin, Wq, Wk, Wv,
                                      Wo, K_cache, V_cache, cache_lens)
    nc = _get_program(nts, rems)

    trace = bool(int(os.environ.get("BASS_KERNEL_TRACE", "0")))
    if trace:
        _install_axon_ntff_hook()
    res = bass_utils.run_bass_kernel_spmd(
        nc, in_maps, core_ids=list(range(N_CORES)), trace=trace)
    _LAST_RESULTS["res"] = res

    total = np.zeros((H, TOK), dtype=np.float64)
    for c in range(N_CORES):
        o = res.results[c]["out"]            # [128, NJT, TOK]
        total += o.transpose(1, 0, 2).reshape(H, TOK).astype(np.float64)
    return total.T.astype(np.float32).reshape(B, Q, H)
